# revision 4
# baseline (speedup 1.0000x reference)
"""Trainium2 Bass kernel for CTRL-style MultiHeadAttention (B=4,S=1024,D=1024,H=16).

Sharding: 8 cores = 4 batches x 2 head-groups. Core c owns batch c//2 and
heads (c%2)*8..(c%2)*8+8 (a 512-wide feature slice of Wq/Wk/Wv columns and
dense rows). Each core computes its 8 heads' attention for its batch and a
partial (512-of-1024 contraction) of the output projection; the host sums
the two partials per batch and re-assembles `present`.

All activations/weights enter the device pre-transposed ([d, t] / [d, f]
layouts packed as [128, chunks, free]) and pre-cast to bf16 by the host, so
the device needs no on-chip transposes:
  qhT/khT [f, t] = WT.T @ xT          (accumulate over d chunks)
  vh      [t, f] = xT_chunk.T @ WvT
  logitsT [k, q] = khT_head.T @ qhT_head          (K = 64)
  ET      = exp(logitsT/8 + attn_mask[k])          (ACT, bias per-partition)
  AV      [65, q] = [vh | 1].T @ ET   (row 64 = softmax denominator)
  mergedT [m, q] = AV[0:64] * bcast(1/AV[64])
  outT    [o, t] partial = dsT.T @ mergedT
head_mask is folded into the dense weight slice rows on the host (exact:
it multiplies post-softmax weights, i.e. scales merged features per head).
"""

import numpy as np
import ml_dtypes

B, S, D, H = 4, 1024, 1024, 16
DEPTH = 64
NCORES = 8
FH = 512  # features (head_dim * heads) per core
BF16 = ml_dtypes.bfloat16

_CACHE = {}
LAST_RESULT = None


def _build_nc():
    from contextlib import ExitStack
    import concourse.tile as tile
    from concourse import bacc, mybir

    f32 = mybir.dt.float32
    bf16 = mybir.dt.bfloat16
    EXP = mybir.ActivationFunctionType.Exp

    nc = bacc.Bacc(None, target_bir_lowering=False)

    # inputs (per-core shards; [128, chunk, free] packings)
    qT = nc.dram_tensor("qt", [128, 8, S], bf16, kind="ExternalInput")
    kT = nc.dram_tensor("kt", [128, 8, S], bf16, kind="ExternalInput")
    vT = nc.dram_tensor("vt", [128, 8, S], bf16, kind="ExternalInput")
    wq = nc.dram_tensor("wq", [128, 8, FH], bf16, kind="ExternalInput")
    wk = nc.dram_tensor("wk", [128, 8, FH], bf16, kind="ExternalInput")
    wv = nc.dram_tensor("wv", [128, 8, FH], bf16, kind="ExternalInput")
    ds = nc.dram_tensor("ds", [128, 4, D], bf16, kind="ExternalInput")
    bq = nc.dram_tensor("bq", [128, 4], f32, kind="ExternalInput")
    bk = nc.dram_tensor("bk", [128, 4], f32, kind="ExternalInput")
    bv = nc.dram_tensor("bv", [128, FH], f32, kind="ExternalInput")
    am = nc.dram_tensor("am", [128, 8], f32, kind="ExternalInput")
    bm = nc.dram_tensor("bm", [128, 128], bf16, kind="ExternalInput")
    # outputs
    outT = nc.dram_tensor("outp", [128, 8, S], f32, kind="ExternalOutput")
    khO = nc.dram_tensor("kho", [128, 4, S], f32, kind="ExternalOutput")
    vhO = nc.dram_tensor("vho", [128, 8, FH], f32, kind="ExternalOutput")

    with tile.TileContext(nc) as tc:
        with ExitStack() as ctx:
            const = ctx.enter_context(tc.tile_pool(name="const", bufs=1))
            work = ctx.enter_context(tc.tile_pool(name="work", bufs=3))
            ppool = ctx.enter_context(tc.tile_pool(name="pp", bufs=2, space="PSUM"))
            lpool = ctx.enter_context(tc.tile_pool(name="lp", bufs=2, space="PSUM"))
            apool = ctx.enter_context(tc.tile_pool(name="ap", bufs=2, space="PSUM"))
            opool = ctx.enter_context(tc.tile_pool(name="op", bufs=2, space="PSUM"))

            # ---- stage inputs into SBUF
            sq = const.tile([128, 8, S], bf16, name="sq")
            sk = const.tile([128, 8, S], bf16, name="sk")
            sv = const.tile([128, 8, S], bf16, name="sv")
            swq = const.tile([128, 8, FH], bf16, name="swq")
            swk = const.tile([128, 8, FH], bf16, name="swk")
            swv = const.tile([128, 8, FH], bf16, name="swv")
            sds = const.tile([128, 4, D], bf16, name="sds")
            sbq = const.tile([128, 4], f32, name="sbq")
            sbk = const.tile([128, 4], f32, name="sbk")
            sbv = const.tile([128, FH], f32, name="sbv")
            sam = const.tile([128, 8], f32, name="sam")
            sbm = const.tile([128, 128], bf16, name="sbm")
            for t_, d_ in [(swk, wk), (sk, kT), (swq, wq), (sq, qT),
                           (swv, wv), (sv, vT), (sds, ds),
                           (sbq, bq), (sbk, bk), (sbv, bv), (sam, am), (sbm, bm)]:
                nc.sync.dma_start(t_[:], d_[:])

            kh32 = const.tile([128, 4, S], f32, name="kh32")
            khb = const.tile([128, 4, S], bf16, name="khb")
            qhb = const.tile([128, 4, S], bf16, name="qhb")
            vh32 = const.tile([128, 8, FH], f32, name="vh32")
            vhb = const.tile([128, 8, 8, 65], bf16, name="vhb")
            mT = const.tile([128, 4, S], bf16, name="mT")

            nc.vector.memset(vhb[:, :, :, 64:65], 1.0)

            # ---- K/Q projections: [f, t] layouts
            for dst32, dstb, w_, x_, b_ in [
                (kh32, khb, swk, sk, sbk),
                (None, qhb, swq, sq, sbq),
            ]:
                for tt in range(2):
                    for fc in range(4):
                        ps = ppool.tile([128, 512], f32, name="psp", tag="psproj")
                        for dc in range(8):
                            nc.tensor.matmul(
                                ps,
                                lhsT=w_[:, dc, fc * 128:(fc + 1) * 128],
                                rhs=x_[:, dc, tt * 512:(tt + 1) * 512],
                                start=(dc == 0),
                                stop=(dc == 7),
                            )
                        if dst32 is not None:
                            o32 = dst32[:, fc, tt * 512:(tt + 1) * 512]
                            nc.vector.tensor_scalar_add(o32, ps, b_[:, fc:fc + 1])
                            nc.vector.tensor_copy(
                                dstb[:, fc, tt * 512:(tt + 1) * 512], o32)
                        else:
                            nc.vector.tensor_scalar_add(
                                dstb[:, fc, tt * 512:(tt + 1) * 512],
                                ps, b_[:, fc:fc + 1])
            nc.sync.dma_start(khO[:], kh32[:])

            # ---- V projection: natural [t, f] layout
            for tcc in range(8):
                ps = ppool.tile([128, 512], f32, name="psv", tag="psproj")
                for dc in range(8):
                    nc.tensor.matmul(
                        ps,
                        lhsT=sv[:, dc, tcc * 128:(tcc + 1) * 128],
                        rhs=swv[:, dc, :],
                        start=(dc == 0),
                        stop=(dc == 7),
                    )
                nc.vector.tensor_add(vh32[:, tcc, :], ps, sbv)
                nc.vector.tensor_copy(
                    vhb[:, tcc, :, 0:64],
                    vh32[:, tcc, :].rearrange("p (h e) -> p h e", h=8))
            nc.sync.dma_start(vhO[:], vh32[:])

            # ---- attention per head
            for h in range(8):
                po = 64 * (h % 2)
                chk = h // 2
                qh_h = qhb[po:po + 64, chk, :]
                kh_h = khb[po:po + 64, chk, :]
                for qt in range(2):
                    kmax = 4 * (qt + 1)
                    et = work.tile([128, 8, 512], bf16, name="et", tag="et", bufs=2)
                    for kc in range(kmax):
                        pl = lpool.tile([128, 512], f32, name="pl", tag="pl")
                        nc.tensor.matmul(
                            pl,
                            lhsT=kh_h[:, kc * 128:(kc + 1) * 128],
                            rhs=qh_h[:, qt * 512:(qt + 1) * 512],
                            start=True, stop=True,
                        )
                        j = kc - 4 * qt
                        lo = max(j, 0) * 128
                        if lo > 0:
                            nc.vector.memset(et[:, kc, 0:lo], 0.0)
                        nc.scalar.activation(
                            et[:, kc, lo:512], pl[:, lo:512], EXP,
                            bias=sam[:, kc:kc + 1], scale=0.125)
                        if j >= 0:
                            # causal diagonal block: keep k <= q
                            nc.vector.tensor_mul(
                                et[:, kc, lo:lo + 128],
                                et[:, kc, lo:lo + 128], sbm)
                    pa = apool.tile([128, 512], f32, name="pa", tag="pa")
                    for kc in range(kmax):
                        nc.tensor.matmul(
                            pa[:65, :],
                            lhsT=vhb[:, kc, h, :],
                            rhs=et[:, kc, :],
                            start=(kc == 0),
                            stop=(kc == kmax - 1),
                        )
                    rr = work.tile([65, 512], f32, name="rr", tag="rr")
                    nc.vector.reciprocal(rr[64:65, :], pa[64:65, :])
                    bc = work.tile([64, 512], f32, name="bc", tag="bc")
                    # partition-broadcast row 64 -> 64 partitions via DMA
                    # (free-dim step-0 replication; DVE cannot cross partitions)
                    nc.sync.dma_start(
                        bc[:], rr[64:65, None, :].to_broadcast([1, 64, 512]))
                    dst = mT[po:po + 64, chk, qt * 512:(qt + 1) * 512]
                    if h % 2 == 0:
                        nc.vector.tensor_mul(dst, pa[0:64, :], bc[:])
                    else:
                        # DVE cannot shift partitions; bounce via SBUF DMA
                        tmp = work.tile([64, 512], bf16, name="tmpod", tag="tmpod")
                        nc.vector.tensor_mul(tmp[:], pa[0:64, :], bc[:])
                        nc.sync.dma_start(dst, tmp[:])

            # ---- output projection (partial over this core's 512 features)
            for tt in range(2):
                for oc in range(8):
                    pd = opool.tile([128, 512], f32, name="pd", tag="pd")
                    for mc in range(4):
                        nc.tensor.matmul(
                            pd,
                            lhsT=sds[:, mc, oc * 128:(oc + 1) * 128],
                            rhs=mT[:, mc, tt * 512:(tt + 1) * 512],
                            start=(mc == 0),
                            stop=(mc == 3),
                        )
                    ot = work.tile([128, 512], f32, name="ot", tag="ot")
                    nc.vector.tensor_copy(ot[:], pd)
                    nc.sync.dma_start(outT[:, oc, tt * 512:(tt + 1) * 512], ot[:])

    return nc


def _get_nc():
    if "nc" not in _CACHE:
        _CACHE["nc"] = _build_nc()
    return _CACHE["nc"]


def _pack(x, chunks):
    """[chunks*128, F] row-major -> [128, chunks, F]."""
    c0 = x.shape[0]
    assert c0 == chunks * 128
    return np.ascontiguousarray(x.reshape(chunks, 128, -1).transpose(1, 0, 2))


def make_in_map(c, inputs):
    b, hg = c // 2, c % 2
    fs = hg * FH
    f32 = np.float32

    def bfT(x):  # transpose then bf16-cast
        return np.ascontiguousarray(np.asarray(x).T).astype(BF16)

    q = np.asarray(inputs["q"][b], f32)
    k = np.asarray(inputs["k"][b], f32)
    v = np.asarray(inputs["v"][b], f32)
    hm = np.asarray(inputs["head_mask"], f32).reshape(H)  # [H]
    hm_cols = np.repeat(hm[hg * 8:(hg + 1) * 8], DEPTH)  # [FH]
    dsl = np.asarray(inputs["dense_w"], f32)[:, fs:fs + FH] * hm_cols[None, :]
    amr = np.asarray(inputs["attention_mask"], f32)[b, 0, 0, :]  # [S]

    return {
        "qt": _pack(bfT(q), 8),
        "kt": _pack(bfT(k), 8),
        "vt": _pack(bfT(v), 8),
        "wq": _pack(bfT(np.asarray(inputs["Wq_w"], f32)[fs:fs + FH, :]), 8),
        "wk": _pack(bfT(np.asarray(inputs["Wk_w"], f32)[fs:fs + FH, :]), 8),
        "wv": _pack(bfT(np.asarray(inputs["Wv_w"], f32)[fs:fs + FH, :]), 8),
        "ds": _pack(bfT(dsl), 4),
        "bq": np.ascontiguousarray(
            np.asarray(inputs["Wq_b"], f32)[fs:fs + FH].reshape(4, 128).T),
        "bk": np.ascontiguousarray(
            np.asarray(inputs["Wk_b"], f32)[fs:fs + FH].reshape(4, 128).T),
        "bv": np.ascontiguousarray(
            np.broadcast_to(np.asarray(inputs["Wv_b"], f32)[fs:fs + FH], (128, FH))),
        "am": np.ascontiguousarray(amr.reshape(8, 128).T),
        "bm": (np.arange(128)[:, None] <= np.arange(128)[None, :]).astype(BF16),
    }


def gather(results, inputs):
    out = np.zeros((B, S, D), np.float32)
    present = np.zeros((2, B, H, S, DEPTH), np.float32)
    for c in range(NCORES):
        b, hg = c // 2, c % 2
        r = results[c]
        oT = r["outp"].transpose(1, 0, 2).reshape(D, S)  # [o, t]
        out[b] += oT.T
        khT = r["kho"].transpose(1, 0, 2).reshape(FH, S)  # [f, t]
        present[0, b, hg * 8:(hg + 1) * 8] = (
            khT.reshape(8, DEPTH, S).transpose(0, 2, 1))
        vh = r["vho"].transpose(1, 0, 2).reshape(S, FH)  # [t, f]
        present[1, b, hg * 8:(hg + 1) * 8] = (
            vh.reshape(S, 8, DEPTH).transpose(1, 0, 2))
    out += np.asarray(inputs["dense_b"], np.float32)[None, None, :]
    return out, present


def kernel(**inputs):
    global LAST_RESULT
    from concourse.bass_utils import run_bass_kernel_spmd

    nc = _get_nc()
    if not nc.is_finalized():
        nc.finalize()  # runs Bacc passes (wait splitting, reg alloc)
    in_maps = [make_in_map(c, inputs) for c in range(NCORES)]
    res = run_bass_kernel_spmd(nc, in_maps, core_ids=list(range(NCORES)))
    LAST_RESULT = res
    return gather(res.results, inputs)
